# revision 3
# baseline (speedup 1.0000x reference)
"""Trainium2 kernel for nn_AxialAttentionBlockAISummer.

Data-parallel over the batch axis across the 8 NeuronCores (one image
per core); weights replicated.  BatchNorm statistics are global: local
(mean, mean-of-squares) moments are combined with cross-core pmean
collectives, so the math matches the single-device reference.

Optimizations vs the naive graph:
 - the joint BN over the concatenated [qr, kr, dots] logits terms is
   folded into per-term per-head affine scales (softmax is invariant to
   the per-row shift, so only the scales are applied) — the [b, 24,
   64, 64] concat tensor is never materialized;
 - the output BN over the stacked [sve, sv] pair is likewise folded
   into per-channel scale/shift applied directly to the two terms;
 - the relative-position embeddings r_q/r_k/r_v (pure gathers of the
   `rel` weight) are precomputed on the host;
 - all per-BN moment pairs are packed into a single flat vector per
   sync point, so each BN costs ONE small AllReduce instead of 2-6
   (the XLA baseline issued 25 collective ops; this issues 6);
 - softmax drops the max-subtraction pass (logits are BN-normalized,
   |logit| is bounded by ~30, exp is safe in fp32);
 - attention einsums run in bf16 with fp32 accumulation; the in/out
   1x1 convs, BN statistics and softmax stay fp32.
"""

import numpy as np

B, C_IN, DIM = 8, 256, 64
HEADS, D_IN, DKQ = 8, 128, 8
DV = D_IN // HEADS            # 16
QKV = 2 * DKQ + DV            # 32
EPS = 1e-5
N_CORES = 8

_compiled = None


def _build(attn_bf16=True):
    import jax
    import jax.numpy as jnp
    from jax.sharding import Mesh, PartitionSpec as P
    try:
        from jax.experimental.shard_map import shard_map
    except ImportError:
        from jax.sharding import shard_map

    devs = jax.devices()[:N_CORES]
    mesh = Mesh(np.asarray(devs), ("b",))
    f32 = jnp.float32
    bf16 = jnp.bfloat16
    mm_dt = bf16 if attn_bf16 else f32

    def mm(spec, a, b):
        return jnp.einsum(spec, a.astype(mm_dt), b.astype(mm_dt),
                          preferred_element_type=f32)

    def mmf(spec, a, b):
        return jnp.einsum(spec, a, b, preferred_element_type=f32)

    def _bn(x, gamma, beta, ch_axis=1):
        """BN with a single fused AllReduce for both moments."""
        axes = tuple(i for i in range(x.ndim) if i != ch_axis)
        m1 = jnp.mean(x, axes)
        m2 = jnp.mean(x * x, axes)
        mom = jax.lax.pmean(jnp.stack([m1, m2], 0), "b")
        m1, m2 = mom[0], mom[1]
        var = m2 - m1 * m1
        shp = [1] * x.ndim
        shp[ch_axis] = -1
        scale = jax.lax.rsqrt(var + EPS) * gamma
        shift = beta - m1 * scale
        return x * scale.reshape(shp) + shift.reshape(shp)

    def _softmax(logits):
        e = jnp.exp(logits)
        return e / jnp.sum(e, axis=-1, keepdims=True)

    def _axial_att(x, w_qkv, rq, rk, rv, ga, ba, go, bo):
        b = x.shape[0]
        qkv = mm("oc,bcd->bod", w_qkv, x)
        qkv = qkv.reshape(b, QKV, HEADS, DIM).transpose(0, 2, 1, 3)
        q = qkv[:, :, :DKQ]
        k = qkv[:, :, DKQ:2 * DKQ]
        v = qkv[:, :, 2 * DKQ:]
        qr = mm("bhid,idj->bhdj", q, rq)
        kr = mm("bhid,idj->bhdj", k, rk)
        dots = mm("bhid,bhij->bhdj", q, k)

        # folded joint BN: logits channel = h*3 + n over (b, d, j); the
        # per-row shift is dropped (softmax is shift-invariant).  One
        # AllReduce for all 6 moment vectors.
        ga3 = ga.reshape(HEADS, 3)
        terms = (qr, kr, dots)
        m1s = jnp.stack([jnp.mean(t, (0, 2, 3)) for t in terms], 0)   # [3,h]
        m2s = jnp.stack([jnp.mean(t * t, (0, 2, 3)) for t in terms], 0)
        mom = jax.lax.pmean(jnp.stack([m1s, m2s], 0), "b")            # [2,3,h]
        logits = 0.
        for n, t in enumerate(terms):
            m1, m2 = mom[0, n], mom[1, n]
            scale = ga3[:, n] * jax.lax.rsqrt(m2 - m1 * m1 + EPS)
            logits = logits + t * scale[None, :, None, None]
        attn = _softmax(logits)

        sv = mm("bhdj,bhij->bhid", attn, v)
        sve = mm("bhdj,idj->bhid", attn, rv)

        # folded output BN: channel = n*D_IN + h*DV + i over (b, d); one
        # AllReduce for all 4 moment tensors.
        go2 = go.reshape(2, HEADS, DV)
        bo2 = bo.reshape(2, HEADS, DV)
        m1s = jnp.stack([jnp.mean(t, (0, 3)) for t in (sve, sv)], 0)  # [2,h,i]
        m2s = jnp.stack([jnp.mean(t * t, (0, 3)) for t in (sve, sv)], 0)
        mom = jax.lax.pmean(jnp.stack([m1s, m2s], 0), "b")            # [2,2,h,i]
        res = 0.
        for n, t in enumerate((sve, sv)):
            m1, m2 = mom[0, n], mom[1, n]
            scale = go2[n] * jax.lax.rsqrt(m2 - m1 * m1 + EPS)
            shift = bo2[n] - m1 * scale
            res = res + t * scale[None, :, :, None] + shift[None, :, :, None]
        return res.reshape(b, D_IN, DIM)

    def fwd(x_in, w_in, g_in, b_in, w_out, g_out, b_out,
            wqkv_h, rq_h, rk_h, rv_h, ga_h, ba_h, go_h, bo_h,
            wqkv_w, rq_w, rk_w, rv_w, ga_w, ba_w, go_w, bo_w):
        bl = x_in.shape[0]
        x = jax.nn.relu(_bn(mm("oc,bchw->bohw", w_in, x_in), g_in, b_in))
        x = x.transpose(0, 3, 1, 2).reshape(bl * DIM, D_IN, DIM)
        x = _axial_att(x, wqkv_h, rq_h, rk_h, rv_h, ga_h, ba_h, go_h, bo_h)
        x = x.reshape(bl, DIM, D_IN, DIM).transpose(0, 3, 2, 1)
        x = x.reshape(bl * DIM, D_IN, DIM)
        x = jax.nn.relu(_axial_att(x, wqkv_w, rq_w, rk_w, rv_w,
                                   ga_w, ba_w, go_w, bo_w))
        x = x.reshape(bl, DIM, D_IN, DIM).transpose(0, 2, 1, 3)
        y = _bn(mm("oc,bchw->bohw", w_out, x), g_out, b_out) + x_in
        return jax.nn.relu(y)

    arg_order = ["x_in", "w_in", "g_in", "b_in", "w_out", "g_out", "b_out",
                 "wqkv_h", "rq_h", "rk_h", "rv_h", "ga_h", "ba_h",
                 "go_h", "bo_h",
                 "wqkv_w", "rq_w", "rk_w", "rv_w", "ga_w", "ba_w",
                 "go_w", "bo_w"]
    in_specs = tuple(P("b") if n == "x_in" else P() for n in arg_order)
    fn = jax.jit(shard_map(fwd, mesh=mesh, in_specs=in_specs,
                           out_specs=P("b"), check_rep=False))
    return fn, arg_order


def _rel_embed(rel):
    """rel [QKV, 2*DIM-1] -> r_q [DKQ,DIM,DIM], r_k [DKQ,DIM,DIM],
    r_v [DV,DIM,DIM] (host-side Toeplitz gather)."""
    idx = (np.arange(DIM)[:, None] - np.arange(DIM)[None, :] + DIM - 1)
    emb = rel[:, idx.reshape(-1)].reshape(QKV, DIM, DIM)
    return emb[:DKQ], emb[DKQ:2 * DKQ], emb[2 * DKQ:]


def kernel(**inputs):
    global _compiled
    if _compiled is None:
        _compiled = _build()
    fn, arg_order = _compiled
    ext = dict(inputs)
    for tag in ("h", "w"):
        rq, rk, rv = _rel_embed(np.asarray(ext["rel_" + tag], np.float32))
        ext["rq_" + tag] = rq
        ext["rk_" + tag] = rk
        ext["rv_" + tag] = rv
    args = [np.asarray(ext[n], np.float32) for n in arg_order]
    out = fn(*args)
    return np.asarray(out, np.float32)
